# revision 22
# baseline (speedup 1.0000x reference)
"""Trainium2 Bass kernel for nn_ComplexAttention (B=8, C=512, H=W=32, HEADS=8).

Strategy
--------
Data-parallel over batch: one batch element per NeuronCore (8 cores), no
collectives.  Host-side algebraic fusion shrinks the per-core work:

  reference:  Q = R_q Wq Z,  K = R_k Wk Z,  V = R_v Wv Z   (complex, [C,T])
              S = Re(Q^H K)/sqrt(dh),  causal softmax -> A
              out = R_o Wo (V A^T)

  fused:      M = Wq^T diag(e^{i(phi_k-phi_q)}) Wk / sqrt(dh)   (host, f64)
              N = diag(e^{i phi_o}) Wo diag(e^{i phi_v}) Wv     (host, f64)
              Y = M Z             (channel-major [C,T])
              St = Re(Y^H Z)      = S^T, computed TRANSPOSED: [u, t]
              Pt = exp(causal(St))          (unnormalized, straight to SBUF)
              l  = colsums(Pt)  (ones-matmul),  R = ones x (1/l)   (PE)
              U = N Z             (token-major [T,C])
              out[:, t] = (U^T Pt) * R      (scale fused into psum->sbuf)

Everything on the PE is bf16 (1 cyc/row at any N), PSUM fp32; outputs are
bf16 and cast back on host.  End-to-end rel err ~8e-3 (budget 2e-2).

Computing S transposed removes all 36 PE transposes + 36 DVE copies of
the softmax path: exp writes the attention tiles Pt[u,t] directly from
PSUM, and out chunk 0 (t<512) is finished and DMA'd mid-kernel.

Schedule notes (from HW traces):
 - single sync DMA queue for the bulk input stream (a concurrent queue
   steals HBM bandwidth from the critical first loads), but the first
   Y_re phase's tiles are split with gpsimd so compute starts ~3us in.
 - psum->sbuf copies alternate vector/scalar except the out copies
   (vector only: they fuse the 1/l column scale via tensor_mul).
"""

import math

import numpy as np

import concourse.mybir as mybir
import concourse.tile as tile
from concourse import bacc
from concourse.bass_utils import run_bass_kernel_spmd

B, C, HH, WW = 8, 512, 32, 32
T = HH * WW          # 1024 tokens
DH = C // 8          # head dim (scale only)
P = 128
CT = C // P          # 4 channel tiles
TT = T // P          # 8 token tiles
NEG = -1.0e30

f32 = mybir.dt.float32
f32r = mybir.dt.float32r
bf16 = mybir.dt.bfloat16
# kept for test.py compat
VALUE_BF16 = True
FULL_BF16 = True


def _mm(nc, out, lhsT, rhs, start, stop):
    nc.tensor.matmul(out, lhsT, rhs, start=start, stop=stop)


_CACHE: dict = {}


def _get_program(has_imag: bool):
    key = has_imag
    if key not in _CACHE:
        _CACHE[key] = _build_program(has_imag)
    return _CACHE[key]


def _build_program(has_imag: bool):
    nc = bacc.Bacc("TRN2", target_bir_lowering=False, debug=False)

    zre_d = nc.dram_tensor("zre", [C, T], bf16, kind="ExternalInput").ap()
    zim_d = nc.dram_tensor("zim", [C, T], bf16, kind="ExternalInput").ap()
    mtre_d = nc.dram_tensor("mtre", [C, C], bf16, kind="ExternalInput").ap()
    ntre_d = nc.dram_tensor("ntre", [C, C], bf16, kind="ExternalInput").ap()
    if has_imag:
        mtim_d = nc.dram_tensor("mtim", [C, C], bf16, kind="ExternalInput").ap()
        mtimn_d = nc.dram_tensor("mtimn", [C, C], bf16, kind="ExternalInput").ap()
        ntim_d = nc.dram_tensor("ntim", [C, C], bf16, kind="ExternalInput").ap()
        ntimn_d = nc.dram_tensor("ntimn", [C, C], bf16, kind="ExternalInput").ap()
    tril_d = nc.dram_tensor("tril", [P, P], f32, kind="ExternalInput").ap()
    onesc_d = nc.dram_tensor("onesc", [P, 1], bf16, kind="ExternalInput").ap()
    onesr_d = nc.dram_tensor("onesr", [1, P], f32r, kind="ExternalInput").ap()
    outre_d = nc.dram_tensor("outre", [C, T], bf16, kind="ExternalOutput").ap()
    outim_d = nc.dram_tensor("outim", [C, T], bf16, kind="ExternalOutput").ap()

    with tile.TileContext(nc) as tc:
        with (
            tc.tile_pool(name="const", bufs=1) as cp,
            tc.tile_pool(name="work", bufs=4) as wp,
            tc.tile_pool(name="small", bufs=12) as sp,
            tc.tile_pool(name="psmm", bufs=6, space="PSUM") as pmm,
            tc.tile_pool(name="psl", bufs=1, space="PSUM") as pl,
            tc.tile_pool(name="psr", bufs=1, space="PSUM") as pr,
        ):
            # -- persistent tiles ------------------------------------------
            tril = cp.tile([P, P], f32, tag="tril", name="tril")
            onesc = cp.tile([P, 1], bf16, tag="onesc", name="onesc")
            onesr = cp.tile([1, P], f32r, tag="onesr", name="onesr")
            mtre = [cp.tile([P, C], bf16, tag=f"mtre{c}", name=f"mtre{c}")
                    for c in range(CT)]
            ntre = [cp.tile([P, C], bf16, tag=f"ntre{c}", name=f"ntre{c}")
                    for c in range(CT)]
            zre_t = [cp.tile([P, T], bf16, tag=f"zre{c}", name=f"zre{c}")
                     for c in range(CT)]
            zim_t = [cp.tile([P, T], bf16, tag=f"zim{c}", name=f"zim{c}")
                     for c in range(CT)]
            zre_h = [[zre_t[c][:, h * 512:(h + 1) * 512] for c in range(CT)]
                     for h in range(2)]
            zim_h = [[zim_t[c][:, h * 512:(h + 1) * 512] for c in range(CT)]
                     for h in range(2)]
            if has_imag:
                mtim = [cp.tile([P, C], bf16, tag=f"mtim{c}", name=f"mtim{c}")
                        for c in range(CT)]
                mtimn = [cp.tile([P, C], bf16, tag=f"mtimn{c}",
                                 name=f"mtimn{c}") for c in range(CT)]
                ntim = [cp.tile([P, C], bf16, tag=f"ntim{c}", name=f"ntim{c}")
                        for c in range(CT)]
                ntimn = [cp.tile([P, C], bf16, tag=f"ntimn{c}",
                                 name=f"ntimn{c}") for c in range(CT)]
            yre = [[cp.tile([P, 512], bf16, tag=f"yre{c}_{n}",
                            name=f"yre{c}_{n}") for n in range(2)]
                   for c in range(CT)]
            yim = [[cp.tile([P, 512], bf16, tag=f"yim{c}_{n}",
                            name=f"yim{c}_{n}") for n in range(2)]
                   for c in range(CT)]
            ure = [cp.tile([P, C], bf16, tag=f"ure{j}", name=f"ure{j}")
                   for j in range(TT)]
            uim = [cp.tile([P, C], bf16, tag=f"uim{j}", name=f"uim{j}")
                   for j in range(TT)]
            # Pt[(j, n)]: exp(S^T) for u-tile j, t-chunk n (written region
            # is cols [max(0, j*P - n*512):512] -- exactly what out reads)
            pt = {}
            for j in range(TT):
                for n in range(2):
                    if n == 0 and j >= 4:
                        continue
                    pt[(j, n)] = cp.tile([P, 512], bf16, tag=f"pt{j}_{n}",
                                         name=f"pt{j}_{n}")

            # -- input DMA: sync bulk stream; gpsimd helps the first phase -
            qA, qC = nc.sync, nc.gpsimd
            qC.dma_start(out=tril, in_=tril_d)
            qC.dma_start(out=onesc, in_=onesc_d)
            qC.dma_start(out=onesr, in_=onesr_d)
            for c in range(CT):
                q = (qA, qC)[c % 2]
                q.dma_start(out=mtre[c], in_=mtre_d[c * P:(c + 1) * P, :])
                q.dma_start(out=zre_t[c], in_=zre_d[c * P:(c + 1) * P, :])
            if has_imag:
                for c in range(CT):
                    qA.dma_start(out=mtimn[c],
                                 in_=mtimn_d[c * P:(c + 1) * P, :])
            for c in range(CT):
                qA.dma_start(out=ntre[c], in_=ntre_d[c * P:(c + 1) * P, :])
            for c in range(CT):
                qA.dma_start(out=zim_t[c], in_=zim_d[c * P:(c + 1) * P, :])
            if has_imag:
                for c in range(CT):
                    qA.dma_start(out=mtim[c],
                                 in_=mtim_d[c * P:(c + 1) * P, :])
                for c in range(CT):
                    qA.dma_start(out=ntim[c],
                                 in_=ntim_d[c * P:(c + 1) * P, :])
                for c in range(CT):
                    qA.dma_start(out=ntimn[c],
                                 in_=ntimn_d[c * P:(c + 1) * P, :])

            def psum_to_sbuf(dst_ap, src_ap, k=0):
                # alternate vector/scalar: scalar is idle until the first
                # softmax exp, so Y/U copies never gate the PE
                if k % 2:
                    nc.scalar.activation(
                        out=dst_ap, in_=src_ap,
                        func=mybir.ActivationFunctionType.Copy)
                else:
                    nc.vector.tensor_copy(out=dst_ap, in_=src_ap)

            def emit_y(dst, terms):
                nterm = len(terms)
                for n in range(2):
                    pss = [pmm.tile([P, 512], f32, tag="mm", name="psmm")
                           for _ in range(CT)]
                    for t_i, (w, zh) in enumerate(terms):
                        for c in range(CT):
                            for m in range(CT):
                                _mm(nc, pss[m], w[c][:, m * P:(m + 1) * P],
                                    zh[n][c],
                                    start=(t_i == 0 and c == 0),
                                    stop=(t_i == nterm - 1 and c == CT - 1))
                    for m in range(CT):
                        psum_to_sbuf(dst[m][n], pss[m], k=m)

            def emit_u(dst, terms):
                for j in range(TT):
                    usl = slice((j % 4) * P, (j % 4 + 1) * P)
                    ps = pmm.tile([P, 512], f32, tag="mm", name="psmm")
                    nacc = len(terms) * CT
                    k = 0
                    for zh, w in terms:
                        for c in range(CT):
                            _mm(nc, ps, zh[j // 4][c][:, usl], w[c][:, :],
                                start=(k == 0), stop=(k == nacc - 1))
                            k += 1
                    psum_to_sbuf(dst[j], ps, k=j)

            if not has_imag:
                emit_y(yre, [(mtre, zre_h)])
                emit_u(ure, [(zre_h, ntre)])
                emit_y(yim, [(mtre, zim_h)])
                emit_u(uim, [(zim_h, ntre)])
            else:
                emit_y(yre, [(mtre, zre_h), (mtimn, zim_h)])
                emit_y(yim, [(mtre, zim_h), (mtim, zre_h)])
                emit_u(ure, [(zre_h, ntre), (zim_h, ntimn)])
                emit_u(uim, [(zim_h, ntre), (zre_h, ntim)])

            def emit_scores_col(j, n):
                """pt[(j,n)] = exp(causal(S^T[u-tile j, t in chunk n]))."""
                jsl = slice((j % 4) * P, (j % 4 + 1) * P)
                lo = max(n * 512, j * P)       # global t start
                hi = (n + 1) * 512
                w = hi - lo
                ps = pmm.tile([P, 512], f32, tag="mm", name="psmm")
                k = 0
                for zt, y in ((zre_t, yre), (zim_t, yim)):
                    for c in range(CT):
                        _mm(nc, ps[:, :w], y[c][j // 4][:, jsl],
                            zt[c][:, lo:hi],
                            start=(k == 0), stop=(k == 2 * CT - 1))
                        k += 1
                dst = pt[(j, n)]
                off = lo - n * 512
                if lo == j * P:
                    # diagonal block: mask t<u, then exp
                    fr = sp.tile([P, P], f32, tag="fr", name="fr")
                    nc.vector.tensor_add(out=fr, in0=ps[:, :P], in1=tril)
                    nc.scalar.activation(
                        out=dst[:, off:off + P], in_=fr,
                        func=mybir.ActivationFunctionType.Exp)
                    if w > P:
                        nc.scalar.activation(
                            out=dst[:, off + P:off + w], in_=ps[:, P:w],
                            func=mybir.ActivationFunctionType.Exp)
                else:
                    nc.scalar.activation(
                        out=dst[:, off:off + w], in_=ps[:, :w],
                        func=mybir.ActivationFunctionType.Exp)

            def emit_l(n):
                """R = ones x (1/colsums(Pt chunk n)): [P,512] in PSUM."""
                js = range(4) if n == 0 else range(TT)
                lps = pl.tile([1, 512], f32, tag="l", name="lps")
                for j in js:
                    lo = max(0, j * P - n * 512)
                    _mm(nc, lps[:, lo:512], onesc, pt[(j, n)][:, lo:512],
                        start=(j == 0), stop=(j == js[-1]))
                rl = sp.tile([1, 512], f32r, tag="rl", name="rl")
                with nc.allow_low_precision(reason="f32r is bit-identical f32"):
                    nc.vector.reciprocal(out=rl, in_=lps)
                rlb = pr.tile([P, 512], f32, tag="rlb", name="rlb")
                _mm(nc, rlb, onesr, rl, start=True, stop=True)
                # tensor_mul can read only one PSUM operand -> stage to SBUF
                rlb_sb = sp.tile([P, 512], f32, tag="rlbsb", name="rlb_sb",
                                 bufs=2)
                nc.vector.tensor_copy(out=rlb_sb, in_=rlb)
                return rlb_sb

            def emit_out_chunk(n, half, rlb):
                """out[:, n*512:(n+1)*512] for re (half 0) / im (half 1)."""
                u, dram = ((ure, outre_d), (uim, outim_d))[half]
                tsl = slice(n * 512, (n + 1) * 512)
                js = list(range(min(4 * n + 4, 8) if n else 4))
                dview = dram.rearrange("(m p) t -> p m t", p=P)
                for mh in range(2):
                    o = wp.tile([P, 2, 512], bf16, tag="osb", name="osb")
                    for mi in range(2):
                        m = 2 * mh + mi
                        msl = slice(m * P, (m + 1) * P)
                        ps = pmm.tile([P, 512], f32, tag="mm", name="psmm")
                        for j in js:
                            lo = max(0, j * P - n * 512)
                            _mm(nc, ps[:, lo:512],
                                u[j][:, msl], pt[(j, n)][:, lo:512],
                                start=(j == js[0]), stop=(j == js[-1]))
                        # fused 1/l column scale during psum->sbuf
                        nc.vector.tensor_mul(out=o[:, mi, :], in0=ps,
                                             in1=rlb)
                    nc.sync.dma_start(out=dview[:, 2 * mh:2 * mh + 2, tsl],
                                      in_=o)

            for j in range(4):
                emit_scores_col(j, 0)
            rlb0 = emit_l(0)
            emit_out_chunk(0, half=0, rlb=rlb0)
            emit_out_chunk(0, half=1, rlb=rlb0)
            for j in range(TT):
                emit_scores_col(j, 1)
            rlb1 = emit_l(1)
            emit_out_chunk(1, half=0, rlb=rlb1)
            emit_out_chunk(1, half=1, rlb=rlb1)

    nc.compile()
    return nc


def _prep_weights(Wq, phi_q, Wk, phi_k, Wv, phi_v, Wo, phi_o):
    Wq, Wk, Wv, Wo = (np.asarray(w, np.float64) for w in (Wq, Wk, Wv, Wo))
    pq, pk, pv, po = (np.asarray(p, np.float64)
                      for p in (phi_q, phi_k, phi_v, phi_o))
    M = (Wq.T @ (np.exp(1j * (pk - pq))[:, None] * Wk)) / math.sqrt(DH)
    N = (np.exp(1j * po)[:, None] * Wo) @ (np.exp(1j * pv)[:, None] * Wv)
    has_imag = not (np.allclose(M.imag, 0.0) and np.allclose(N.imag, 0.0))
    return M, N, has_imag


def _consts(has_imag, M, N):
    import ml_dtypes
    snp = ml_dtypes.bfloat16
    consts = {
        "mtre": np.ascontiguousarray(M.real.T.astype(snp)),
        "ntre": np.ascontiguousarray(N.real.T.astype(snp)),
        "tril": np.tril(np.full((P, P), NEG, np.float32), -1),
        "onesc": np.ones((P, 1), snp),
        "onesr": np.ones((1, P), np.float32),
    }
    if has_imag:
        mtim = np.ascontiguousarray(M.imag.T.astype(snp))
        ntim = np.ascontiguousarray(N.imag.T.astype(snp))
        consts.update(mtim=mtim, mtimn=-mtim, ntim=ntim, ntimn=-ntim)
    return consts


def kernel(z_re, z_im, Wq, phi_q, Wk, phi_k, Wv, phi_v, Wo, phi_o):
    import ml_dtypes
    snp = ml_dtypes.bfloat16
    z_re = np.ascontiguousarray(np.asarray(z_re, np.float32).astype(snp))
    z_im = np.ascontiguousarray(np.asarray(z_im, np.float32).astype(snp))
    M, N, has_imag = _prep_weights(Wq, phi_q, Wk, phi_k, Wv, phi_v, Wo, phi_o)
    consts = _consts(has_imag, M, N)

    nc = _get_program(has_imag)
    in_maps = [
        dict(consts, zre=z_re[b].reshape(C, T), zim=z_im[b].reshape(C, T))
        for b in range(B)
    ]
    res = run_bass_kernel_spmd(nc, in_maps, list(range(B)))
    out_re = np.stack([np.asarray(res.results[b]["outre"], np.float32)
                       .reshape(C, HH, WW) for b in range(B)])
    out_im = np.stack([np.asarray(res.results[b]["outim"], np.float32)
                       .reshape(C, HH, WW) for b in range(B)])
    return out_re, out_im


# revision 26
# speedup vs baseline: 1.0508x; 1.0508x over previous
"""Trainium2 Bass kernel for nn_ComplexAttention (B=8, C=512, H=W=32, HEADS=8).

Strategy
--------
Data-parallel over batch: one batch element per NeuronCore (8 cores), no
collectives.  Host-side algebraic fusion shrinks the per-core work:

  reference:  Q = R_q Wq Z,  K = R_k Wk Z,  V = R_v Wv Z   (complex, [C,T])
              S = Re(Q^H K)/sqrt(dh),  causal softmax -> A
              out = R_o Wo (V A^T)

  fused:      M = Wq^T diag(e^{i(phi_k-phi_q)}) Wk / sqrt(dh)   (host, f64)
              N = diag(e^{i phi_o}) Wo diag(e^{i phi_v}) Wv     (host, f64)
              Y = M Z             (channel-major [C,T])
              St = Re(Y^H Z)      = S^T, computed TRANSPOSED: [u, t]
              Pt = exp(causal(St))          (unnormalized, straight to SBUF)
              l  = colsums(Pt)  (ones-matmul),  R = ones x (1/l)   (PE)
              U = N Z             (token-major [T,C])
              out[:, t] = (U^T Pt) * R      (scale fused into psum->sbuf)

Everything on the PE is bf16 (1 cyc/row at any N), PSUM fp32; outputs are
bf16 and cast back on host.  End-to-end rel err ~8e-3 (budget 2e-2).

Computing S transposed removes all 36 PE transposes + 36 DVE copies of
the softmax path: exp writes the attention tiles Pt[u,t] directly from
PSUM, and out chunk 0 (t<512) is finished and DMA'd mid-kernel.

Schedule notes (from HW traces):
 - single sync DMA queue for the bulk input stream (a concurrent queue
   steals HBM bandwidth from the critical first loads), but the first
   Y_re phase's tiles are split with gpsimd so compute starts ~3us in.
 - psum->sbuf copies alternate vector/scalar except the out copies
   (vector only: they fuse the 1/l column scale via tensor_mul).
"""

import math

import numpy as np

import concourse.mybir as mybir
import concourse.tile as tile
from concourse import bacc
from concourse.bass_utils import run_bass_kernel_spmd

B, C, HH, WW = 8, 512, 32, 32
T = HH * WW          # 1024 tokens
DH = C // 8          # head dim (scale only)
P = 128
CT = C // P          # 4 channel tiles
TT = T // P          # 8 token tiles
NEG = -1.0e30

f32 = mybir.dt.float32
f32r = mybir.dt.float32r
bf16 = mybir.dt.bfloat16
# kept for test.py compat
VALUE_BF16 = True
FULL_BF16 = True


def _mm(nc, out, lhsT, rhs, start, stop):
    nc.tensor.matmul(out, lhsT, rhs, start=start, stop=stop)


_CACHE: dict = {}


def _get_program(has_imag: bool):
    key = has_imag
    if key not in _CACHE:
        _CACHE[key] = _build_program(has_imag)
    return _CACHE[key]


def _build_program(has_imag: bool):
    nc = bacc.Bacc("TRN2", target_bir_lowering=False, debug=False)

    zre_d = nc.dram_tensor("zre", [C, T], bf16, kind="ExternalInput").ap()
    zim_d = nc.dram_tensor("zim", [C, T], bf16, kind="ExternalInput").ap()
    mtre_d = nc.dram_tensor("mtre", [C, C], bf16, kind="ExternalInput").ap()
    ntre_d = nc.dram_tensor("ntre", [C, C], bf16, kind="ExternalInput").ap()
    if has_imag:
        mtim_d = nc.dram_tensor("mtim", [C, C], bf16, kind="ExternalInput").ap()
        mtimn_d = nc.dram_tensor("mtimn", [C, C], bf16, kind="ExternalInput").ap()
        ntim_d = nc.dram_tensor("ntim", [C, C], bf16, kind="ExternalInput").ap()
        ntimn_d = nc.dram_tensor("ntimn", [C, C], bf16, kind="ExternalInput").ap()
    tril_d = nc.dram_tensor("tril", [P, P], f32, kind="ExternalInput").ap()
    onesc_d = nc.dram_tensor("onesc", [P, 1], bf16, kind="ExternalInput").ap()
    onesr_d = nc.dram_tensor("onesr", [1, P], f32r, kind="ExternalInput").ap()
    outre_d = nc.dram_tensor("outre", [C, T], bf16, kind="ExternalOutput").ap()
    outim_d = nc.dram_tensor("outim", [C, T], bf16, kind="ExternalOutput").ap()

    with tile.TileContext(nc) as tc:
        with (
            tc.tile_pool(name="const", bufs=1) as cp,
            tc.tile_pool(name="work", bufs=4) as wp,
            tc.tile_pool(name="small", bufs=12) as sp,
            tc.tile_pool(name="psmm", bufs=6, space="PSUM") as pmm,
            tc.tile_pool(name="psl", bufs=1, space="PSUM") as pl,
            tc.tile_pool(name="psr", bufs=1, space="PSUM") as pr,
        ):
            # -- persistent tiles ------------------------------------------
            tril = cp.tile([P, P], f32, tag="tril", name="tril")
            onesc = cp.tile([P, 1], bf16, tag="onesc", name="onesc")
            onesr = cp.tile([1, P], f32r, tag="onesr", name="onesr")
            mtre = [cp.tile([P, C], bf16, tag=f"mtre{c}", name=f"mtre{c}")
                    for c in range(CT)]
            ntre = [cp.tile([P, C], bf16, tag=f"ntre{c}", name=f"ntre{c}")
                    for c in range(CT)]
            zre_t = [cp.tile([P, T], bf16, tag=f"zre{c}", name=f"zre{c}")
                     for c in range(CT)]
            zim_t = [cp.tile([P, T], bf16, tag=f"zim{c}", name=f"zim{c}")
                     for c in range(CT)]
            zre_h = [[zre_t[c][:, h * 512:(h + 1) * 512] for c in range(CT)]
                     for h in range(2)]
            zim_h = [[zim_t[c][:, h * 512:(h + 1) * 512] for c in range(CT)]
                     for h in range(2)]
            if has_imag:
                mtim = [cp.tile([P, C], bf16, tag=f"mtim{c}", name=f"mtim{c}")
                        for c in range(CT)]
                mtimn = [cp.tile([P, C], bf16, tag=f"mtimn{c}",
                                 name=f"mtimn{c}") for c in range(CT)]
                ntim = [cp.tile([P, C], bf16, tag=f"ntim{c}", name=f"ntim{c}")
                        for c in range(CT)]
                ntimn = [cp.tile([P, C], bf16, tag=f"ntimn{c}",
                                 name=f"ntimn{c}") for c in range(CT)]
            yre = [[cp.tile([P, 512], bf16, tag=f"yre{c}_{n}",
                            name=f"yre{c}_{n}") for n in range(2)]
                   for c in range(CT)]
            yim = [[cp.tile([P, 512], bf16, tag=f"yim{c}_{n}",
                            name=f"yim{c}_{n}") for n in range(2)]
                   for c in range(CT)]
            ure = [cp.tile([P, C], bf16, tag=f"ure{j}", name=f"ure{j}")
                   for j in range(TT)]
            uim = [cp.tile([P, C], bf16, tag=f"uim{j}", name=f"uim{j}")
                   for j in range(TT)]
            # Pt[(j, n)]: exp(S^T) for u-tile j, t-chunk n (written region
            # is cols [max(0, j*P - n*512):512] -- exactly what out reads)
            pt = {}
            for j in range(TT):
                for n in range(2):
                    if n == 0 and j >= 4:
                        continue
                    pt[(j, n)] = cp.tile([P, 512], bf16, tag=f"pt{j}_{n}",
                                         name=f"pt{j}_{n}")

            # -- input DMA: sync bulk stream; gpsimd helps the first phase -
            qA, qC = nc.sync, nc.gpsimd
            qC.dma_start(out=tril, in_=tril_d)
            qC.dma_start(out=onesc, in_=onesc_d)
            qC.dma_start(out=onesr, in_=onesr_d)
            for c in range(CT):
                qA.dma_start(out=mtre[c], in_=mtre_d[c * P:(c + 1) * P, :])
                qA.dma_start(out=zre_t[c], in_=zre_d[c * P:(c + 1) * P, :])
            if has_imag:
                for c in range(CT):
                    qA.dma_start(out=mtimn[c],
                                 in_=mtimn_d[c * P:(c + 1) * P, :])
            for c in range(CT):
                qA.dma_start(out=ntre[c], in_=ntre_d[c * P:(c + 1) * P, :])
            for c in range(CT):
                qA.dma_start(out=zim_t[c], in_=zim_d[c * P:(c + 1) * P, :])
            if has_imag:
                for c in range(CT):
                    qA.dma_start(out=mtim[c],
                                 in_=mtim_d[c * P:(c + 1) * P, :])
                for c in range(CT):
                    qA.dma_start(out=ntim[c],
                                 in_=ntim_d[c * P:(c + 1) * P, :])
                for c in range(CT):
                    qA.dma_start(out=ntimn[c],
                                 in_=ntimn_d[c * P:(c + 1) * P, :])

            def psum_to_sbuf(dst_ap, src_ap, k=0):
                # alternate vector/scalar: scalar is idle until the first
                # softmax exp, so Y/U copies never gate the PE
                if k % 2:
                    nc.scalar.activation(
                        out=dst_ap, in_=src_ap,
                        func=mybir.ActivationFunctionType.Copy)
                else:
                    nc.vector.tensor_copy(out=dst_ap, in_=src_ap)

            def emit_y(dst, terms):
                nterm = len(terms)
                for n in range(2):
                    pss = [pmm.tile([P, 512], f32, tag="mm", name="psmm")
                           for _ in range(CT)]
                    for t_i, (w, zh) in enumerate(terms):
                        for c in range(CT):
                            for m in range(CT):
                                _mm(nc, pss[m], w[c][:, m * P:(m + 1) * P],
                                    zh[n][c],
                                    start=(t_i == 0 and c == 0),
                                    stop=(t_i == nterm - 1 and c == CT - 1))
                    for m in range(CT):
                        psum_to_sbuf(dst[m][n], pss[m], k=m)

            def emit_u(dst, terms):
                for j in range(TT):
                    usl = slice((j % 4) * P, (j % 4 + 1) * P)
                    ps = pmm.tile([P, 512], f32, tag="mm", name="psmm")
                    nacc = len(terms) * CT
                    k = 0
                    for zh, w in terms:
                        for c in range(CT):
                            _mm(nc, ps, zh[j // 4][c][:, usl], w[c][:, :],
                                start=(k == 0), stop=(k == nacc - 1))
                            k += 1
                    psum_to_sbuf(dst[j], ps, k=j)

            if not has_imag:
                emit_y(yre, [(mtre, zre_h)])
                emit_u(ure, [(zre_h, ntre)])
                emit_y(yim, [(mtre, zim_h)])
                emit_u(uim, [(zim_h, ntre)])
            else:
                emit_y(yre, [(mtre, zre_h), (mtimn, zim_h)])
                emit_y(yim, [(mtre, zim_h), (mtim, zre_h)])
                emit_u(ure, [(zre_h, ntre), (zim_h, ntimn)])
                emit_u(uim, [(zim_h, ntre), (zre_h, ntim)])

            def emit_scores_col(j, n):
                """pt[(j,n)] = exp(causal(S^T[u-tile j, t in chunk n]))."""
                jsl = slice((j % 4) * P, (j % 4 + 1) * P)
                lo = max(n * 512, j * P)       # global t start
                hi = (n + 1) * 512
                w = hi - lo
                ps = pmm.tile([P, 512], f32, tag="mm", name="psmm")
                k = 0
                for zt, y in ((zre_t, yre), (zim_t, yim)):
                    for c in range(CT):
                        _mm(nc, ps[:, :w], y[c][j // 4][:, jsl],
                            zt[c][:, lo:hi],
                            start=(k == 0), stop=(k == 2 * CT - 1))
                        k += 1
                dst = pt[(j, n)]
                off = lo - n * 512
                if lo == j * P:
                    # diagonal block: mask t<u, then exp
                    fr = sp.tile([P, P], f32, tag="fr", name="fr")
                    nc.vector.tensor_add(out=fr, in0=ps[:, :P], in1=tril)
                    nc.scalar.activation(
                        out=dst[:, off:off + P], in_=fr,
                        func=mybir.ActivationFunctionType.Exp)
                    if w > P:
                        nc.scalar.activation(
                            out=dst[:, off + P:off + w], in_=ps[:, P:w],
                            func=mybir.ActivationFunctionType.Exp)
                else:
                    nc.scalar.activation(
                        out=dst[:, off:off + w], in_=ps[:, :w],
                        func=mybir.ActivationFunctionType.Exp)

            def emit_l_mm(lps, j, n, njs):
                """one colsum accumulation step for Pt[(j,n)]"""
                lo = max(0, j * P - n * 512)
                _mm(nc, lps[:, lo:512], onesc, pt[(j, n)][:, lo:512],
                    start=(j == 0), stop=(j == njs - 1))

            def emit_rlb(lps):
                """R = ones x (1/l): [P,512], staged to SBUF."""
                rl = sp.tile([1, 512], f32r, tag="rl", name="rl")
                with nc.allow_low_precision(reason="f32r is bit-identical f32"):
                    nc.vector.reciprocal(out=rl, in_=lps)
                rlb = pr.tile([P, 512], f32, tag="rlb", name="rlb")
                _mm(nc, rlb, onesr, rl, start=True, stop=True)
                # tensor_mul can read only one PSUM operand -> stage to SBUF
                rlb_sb = sp.tile([P, 512], f32, tag="rlbsb", name="rlb_sb",
                                 bufs=2)
                nc.vector.tensor_copy(out=rlb_sb, in_=rlb)
                return rlb_sb

            def emit_out_chunk(n, half, rlb):
                """out[:, n*512:(n+1)*512] for re (half 0) / im (half 1)."""
                u, dram = ((ure, outre_d), (uim, outim_d))[half]
                tsl = slice(n * 512, (n + 1) * 512)
                js = list(range(min(4 * n + 4, 8) if n else 4))
                dview = dram.rearrange("(m p) t -> p m t", p=P)
                for mh in range(2):
                    o = wp.tile([P, 2, 512], bf16, tag="osb", name="osb")
                    for mi in range(2):
                        m = 2 * mh + mi
                        msl = slice(m * P, (m + 1) * P)
                        ps = pmm.tile([P, 512], f32, tag="mm", name="psmm")
                        for j in js:
                            lo = max(0, j * P - n * 512)
                            _mm(nc, ps[:, lo:512],
                                u[j][:, msl], pt[(j, n)][:, lo:512],
                                start=(j == js[0]), stop=(j == js[-1]))
                        # fused 1/l column scale during psum->sbuf
                        nc.vector.tensor_mul(out=o[:, mi, :], in0=ps,
                                             in1=rlb)
                    nc.sync.dma_start(out=dview[:, 2 * mh:2 * mh + 2, tsl],
                                      in_=o)

            # l-matmul for column j is emitted after scores-col j+1 so its
            # wait on exp(j) is always covered by independent matmuls
            lps0 = pl.tile([1, 512], f32, tag="l", name="lps")
            emit_scores_col(0, 0)
            emit_scores_col(1, 0)
            emit_l_mm(lps0, 0, 0, 4)
            emit_scores_col(2, 0)
            emit_l_mm(lps0, 1, 0, 4)
            emit_scores_col(3, 0)
            emit_l_mm(lps0, 2, 0, 4)
            emit_scores_col(0, 1)
            emit_l_mm(lps0, 3, 0, 4)
            emit_scores_col(1, 1)
            rlb0 = emit_rlb(lps0)
            emit_out_chunk(0, half=0, rlb=rlb0)
            emit_out_chunk(0, half=1, rlb=rlb0)
            lps1 = pl.tile([1, 512], f32, tag="l", name="lps")
            emit_scores_col(2, 1)
            emit_l_mm(lps1, 0, 1, 8)
            emit_scores_col(3, 1)
            emit_l_mm(lps1, 1, 1, 8)
            emit_scores_col(4, 1)
            emit_l_mm(lps1, 2, 1, 8)
            emit_scores_col(5, 1)
            emit_l_mm(lps1, 3, 1, 8)
            emit_scores_col(6, 1)
            emit_l_mm(lps1, 4, 1, 8)
            emit_scores_col(7, 1)
            emit_l_mm(lps1, 5, 1, 8)
            emit_l_mm(lps1, 6, 1, 8)
            emit_l_mm(lps1, 7, 1, 8)
            rlb1 = emit_rlb(lps1)
            emit_out_chunk(1, half=0, rlb=rlb1)
            emit_out_chunk(1, half=1, rlb=rlb1)

    nc.compile()
    return nc


def _prep_weights(Wq, phi_q, Wk, phi_k, Wv, phi_v, Wo, phi_o):
    Wq, Wk, Wv, Wo = (np.asarray(w, np.float64) for w in (Wq, Wk, Wv, Wo))
    pq, pk, pv, po = (np.asarray(p, np.float64)
                      for p in (phi_q, phi_k, phi_v, phi_o))
    M = (Wq.T @ (np.exp(1j * (pk - pq))[:, None] * Wk)) / math.sqrt(DH)
    N = (np.exp(1j * po)[:, None] * Wo) @ (np.exp(1j * pv)[:, None] * Wv)
    has_imag = not (np.allclose(M.imag, 0.0) and np.allclose(N.imag, 0.0))
    return M, N, has_imag


def _consts(has_imag, M, N):
    import ml_dtypes
    snp = ml_dtypes.bfloat16
    consts = {
        "mtre": np.ascontiguousarray(M.real.T.astype(snp)),
        "ntre": np.ascontiguousarray(N.real.T.astype(snp)),
        "tril": np.tril(np.full((P, P), NEG, np.float32), -1),
        "onesc": np.ones((P, 1), snp),
        "onesr": np.ones((1, P), np.float32),
    }
    if has_imag:
        mtim = np.ascontiguousarray(M.imag.T.astype(snp))
        ntim = np.ascontiguousarray(N.imag.T.astype(snp))
        consts.update(mtim=mtim, mtimn=-mtim, ntim=ntim, ntimn=-ntim)
    return consts


def kernel(z_re, z_im, Wq, phi_q, Wk, phi_k, Wv, phi_v, Wo, phi_o):
    import ml_dtypes
    snp = ml_dtypes.bfloat16
    z_re = np.ascontiguousarray(np.asarray(z_re, np.float32).astype(snp))
    z_im = np.ascontiguousarray(np.asarray(z_im, np.float32).astype(snp))
    M, N, has_imag = _prep_weights(Wq, phi_q, Wk, phi_k, Wv, phi_v, Wo, phi_o)
    consts = _consts(has_imag, M, N)

    nc = _get_program(has_imag)
    in_maps = [
        dict(consts, zre=z_re[b].reshape(C, T), zim=z_im[b].reshape(C, T))
        for b in range(B)
    ]
    res = run_bass_kernel_spmd(nc, in_maps, list(range(B)))
    out_re = np.stack([np.asarray(res.results[b]["outre"], np.float32)
                       .reshape(C, HH, WW) for b in range(B)])
    out_im = np.stack([np.asarray(res.results[b]["outim"], np.float32)
                       .reshape(C, HH, WW) for b in range(B)])
    return out_re, out_im


# revision 29
# speedup vs baseline: 1.0780x; 1.0259x over previous
"""Trainium2 Bass kernel for nn_ComplexAttention (B=8, C=512, H=W=32, HEADS=8).

Strategy
--------
Data-parallel over batch: one batch element per NeuronCore (8 cores), no
collectives.  Host-side algebraic fusion shrinks the per-core work:

  reference:  Q = R_q Wq Z,  K = R_k Wk Z,  V = R_v Wv Z   (complex, [C,T])
              S = Re(Q^H K)/sqrt(dh),  causal softmax -> A
              out = R_o Wo (V A^T)

  fused:      M = Wq^T diag(e^{i(phi_k-phi_q)}) Wk / sqrt(dh)   (host, f64)
              N = diag(e^{i phi_o}) Wo diag(e^{i phi_v}) Wv     (host, f64)
              Y = M Z             (channel-major [C,T])
              St = Re(Y^H Z)      = S^T, computed TRANSPOSED: [u, t]
              Pt = exp(causal(St))          (unnormalized, straight to SBUF)
              l  = colsums(Pt)  (ones-matmul),  R = ones x (1/l)   (PE)
              U = N Z             (token-major [T,C])
              out[:, t] = (U^T Pt) * R      (scale fused into psum->sbuf)

Everything on the PE is bf16 (1 cyc/row at any N), PSUM fp32; outputs are
bf16 and cast back on host.  End-to-end rel err ~8e-3 (budget 2e-2).

Computing S transposed removes all 36 PE transposes + 36 DVE copies of
the softmax path: exp writes the attention tiles Pt[u,t] directly from
PSUM, and out chunk 0 (t<512) is finished and DMA'd mid-kernel.

Schedule notes (from HW traces):
 - single sync DMA queue for the bulk input stream (a concurrent queue
   steals HBM bandwidth from the critical first loads), but the first
   Y_re phase's tiles are split with gpsimd so compute starts ~3us in.
 - psum->sbuf copies alternate vector/scalar except the out copies
   (vector only: they fuse the 1/l column scale via tensor_mul).
"""

import math

import numpy as np

import concourse.mybir as mybir
import concourse.tile as tile
from concourse import bacc
from concourse.bass_utils import run_bass_kernel_spmd

B, C, HH, WW = 8, 512, 32, 32
T = HH * WW          # 1024 tokens
DH = C // 8          # head dim (scale only)
P = 128
CT = C // P          # 4 channel tiles
TT = T // P          # 8 token tiles
NEG = -1.0e30

f32 = mybir.dt.float32
f32r = mybir.dt.float32r
bf16 = mybir.dt.bfloat16
# kept for test.py compat
VALUE_BF16 = True
FULL_BF16 = True


def _mm(nc, out, lhsT, rhs, start, stop):
    nc.tensor.matmul(out, lhsT, rhs, start=start, stop=stop)


_CACHE: dict = {}


def _get_program(has_imag: bool):
    key = has_imag
    if key not in _CACHE:
        _CACHE[key] = _build_program(has_imag)
    return _CACHE[key]


def _build_program(has_imag: bool):
    nc = bacc.Bacc("TRN2", target_bir_lowering=False, debug=False)

    zre_d = nc.dram_tensor("zre", [C, T], bf16, kind="ExternalInput").ap()
    zim_d = nc.dram_tensor("zim", [C, T], bf16, kind="ExternalInput").ap()
    mtre_d = nc.dram_tensor("mtre", [C, C], bf16, kind="ExternalInput").ap()
    ntre_d = nc.dram_tensor("ntre", [C, C], bf16, kind="ExternalInput").ap()
    if has_imag:
        mtim_d = nc.dram_tensor("mtim", [C, C], bf16, kind="ExternalInput").ap()
        mtimn_d = nc.dram_tensor("mtimn", [C, C], bf16, kind="ExternalInput").ap()
        ntim_d = nc.dram_tensor("ntim", [C, C], bf16, kind="ExternalInput").ap()
        ntimn_d = nc.dram_tensor("ntimn", [C, C], bf16, kind="ExternalInput").ap()
    tril_d = nc.dram_tensor("tril", [P, P], f32, kind="ExternalInput").ap()
    onesc_d = nc.dram_tensor("onesc", [P, 1], bf16, kind="ExternalInput").ap()
    onesr_d = nc.dram_tensor("onesr", [1, P], f32r, kind="ExternalInput").ap()
    outre_d = nc.dram_tensor("outre", [C, T], bf16, kind="ExternalOutput").ap()
    outim_d = nc.dram_tensor("outim", [C, T], bf16, kind="ExternalOutput").ap()

    with tile.TileContext(nc) as tc:
        with (
            tc.tile_pool(name="const", bufs=1) as cp,
            tc.tile_pool(name="work", bufs=4) as wp,
            tc.tile_pool(name="small", bufs=12) as sp,
            tc.tile_pool(name="psmm", bufs=6, space="PSUM") as pmm,
            tc.tile_pool(name="psl", bufs=1, space="PSUM") as pl,
            tc.tile_pool(name="psr", bufs=1, space="PSUM") as pr,
        ):
            # -- persistent tiles ------------------------------------------
            tril = cp.tile([P, P], f32, tag="tril", name="tril")
            onesc = cp.tile([P, 1], bf16, tag="onesc", name="onesc")
            onesr = cp.tile([1, P], f32r, tag="onesr", name="onesr")
            mtre = [cp.tile([P, C], bf16, tag=f"mtre{c}", name=f"mtre{c}")
                    for c in range(CT)]
            ntre = [cp.tile([P, C], bf16, tag=f"ntre{c}", name=f"ntre{c}")
                    for c in range(CT)]
            zre_t = [cp.tile([P, T], bf16, tag=f"zre{c}", name=f"zre{c}")
                     for c in range(CT)]
            zim_t = [cp.tile([P, T], bf16, tag=f"zim{c}", name=f"zim{c}")
                     for c in range(CT)]
            zre_h = [[zre_t[c][:, h * 512:(h + 1) * 512] for c in range(CT)]
                     for h in range(2)]
            zim_h = [[zim_t[c][:, h * 512:(h + 1) * 512] for c in range(CT)]
                     for h in range(2)]
            if has_imag:
                mtim = [cp.tile([P, C], bf16, tag=f"mtim{c}", name=f"mtim{c}")
                        for c in range(CT)]
                mtimn = [cp.tile([P, C], bf16, tag=f"mtimn{c}",
                                 name=f"mtimn{c}") for c in range(CT)]
                ntim = [cp.tile([P, C], bf16, tag=f"ntim{c}", name=f"ntim{c}")
                        for c in range(CT)]
                ntimn = [cp.tile([P, C], bf16, tag=f"ntimn{c}",
                                 name=f"ntimn{c}") for c in range(CT)]
            yre = [[cp.tile([P, 512], bf16, tag=f"yre{c}_{n}",
                            name=f"yre{c}_{n}") for n in range(2)]
                   for c in range(CT)]
            yim = [[cp.tile([P, 512], bf16, tag=f"yim{c}_{n}",
                            name=f"yim{c}_{n}") for n in range(2)]
                   for c in range(CT)]
            ure = [cp.tile([P, C], bf16, tag=f"ure{j}", name=f"ure{j}")
                   for j in range(TT)]
            uim = [cp.tile([P, C], bf16, tag=f"uim{j}", name=f"uim{j}")
                   for j in range(TT)]
            # Pt[(j, n)]: exp(S^T) for u-tile j, t-chunk n (written region
            # is cols [max(0, j*P - n*512):512] -- exactly what out reads)
            pt = {}
            for j in range(TT):
                for n in range(2):
                    if n == 0 and j >= 4:
                        continue
                    pt[(j, n)] = cp.tile([P, 512], bf16, tag=f"pt{j}_{n}",
                                         name=f"pt{j}_{n}")

            # -- input DMA: sync bulk stream; gpsimd helps the first phase -
            qA, qC = nc.sync, nc.gpsimd
            qC.dma_start(out=tril, in_=tril_d)
            qC.dma_start(out=onesc, in_=onesc_d)
            qC.dma_start(out=onesr, in_=onesr_d)
            # halves loaded separately: consumers of h0 don't wait for h1
            for c in range(CT):
                qA.dma_start(out=mtre[c], in_=mtre_d[c * P:(c + 1) * P, :])
                qA.dma_start(out=zre_h[0][c],
                             in_=zre_d[c * P:(c + 1) * P, 0:512])
            for c in range(CT):
                qA.dma_start(out=zre_h[1][c],
                             in_=zre_d[c * P:(c + 1) * P, 512:1024])
            if has_imag:
                for c in range(CT):
                    qA.dma_start(out=mtimn[c],
                                 in_=mtimn_d[c * P:(c + 1) * P, :])
            for c in range(CT):
                qA.dma_start(out=ntre[c], in_=ntre_d[c * P:(c + 1) * P, :])
            for c in range(CT):
                qA.dma_start(out=zim_h[0][c],
                             in_=zim_d[c * P:(c + 1) * P, 0:512])
            for c in range(CT):
                qA.dma_start(out=zim_h[1][c],
                             in_=zim_d[c * P:(c + 1) * P, 512:1024])
            if has_imag:
                for c in range(CT):
                    qA.dma_start(out=mtim[c],
                                 in_=mtim_d[c * P:(c + 1) * P, :])
                for c in range(CT):
                    qA.dma_start(out=ntim[c],
                                 in_=ntim_d[c * P:(c + 1) * P, :])
                for c in range(CT):
                    qA.dma_start(out=ntimn[c],
                                 in_=ntimn_d[c * P:(c + 1) * P, :])

            def psum_to_sbuf(dst_ap, src_ap, k=0):
                # alternate vector/scalar: scalar is idle until the first
                # softmax exp, so Y/U copies never gate the PE
                if k % 2:
                    nc.scalar.activation(
                        out=dst_ap, in_=src_ap,
                        func=mybir.ActivationFunctionType.Copy)
                else:
                    nc.vector.tensor_copy(out=dst_ap, in_=src_ap)

            def emit_y(dst, terms):
                nterm = len(terms)
                for n in range(2):
                    pss = [pmm.tile([P, 512], f32, tag="mm", name="psmm")
                           for _ in range(CT)]
                    for t_i, (w, zh) in enumerate(terms):
                        for c in range(CT):
                            for m in range(CT):
                                _mm(nc, pss[m], w[c][:, m * P:(m + 1) * P],
                                    zh[n][c],
                                    start=(t_i == 0 and c == 0),
                                    stop=(t_i == nterm - 1 and c == CT - 1))
                    for m in range(CT):
                        psum_to_sbuf(dst[m][n], pss[m], k=m)

            def emit_u(dst, terms):
                for j in range(TT):
                    usl = slice((j % 4) * P, (j % 4 + 1) * P)
                    ps = pmm.tile([P, 512], f32, tag="mm", name="psmm")
                    nacc = len(terms) * CT
                    k = 0
                    for zh, w in terms:
                        for c in range(CT):
                            _mm(nc, ps, zh[j // 4][c][:, usl], w[c][:, :],
                                start=(k == 0), stop=(k == nacc - 1))
                            k += 1
                    psum_to_sbuf(dst[j], ps, k=j)

            if not has_imag:
                emit_y(yre, [(mtre, zre_h)])
                emit_u(ure, [(zre_h, ntre)])
                emit_y(yim, [(mtre, zim_h)])
                emit_u(uim, [(zim_h, ntre)])
            else:
                emit_y(yre, [(mtre, zre_h), (mtimn, zim_h)])
                emit_y(yim, [(mtre, zim_h), (mtim, zre_h)])
                emit_u(ure, [(zre_h, ntre), (zim_h, ntimn)])
                emit_u(uim, [(zim_h, ntre), (zre_h, ntim)])

            def emit_scores_col(j, n):
                """pt[(j,n)] = exp(causal(S^T[u-tile j, t in chunk n]))."""
                jsl = slice((j % 4) * P, (j % 4 + 1) * P)
                lo = max(n * 512, j * P)       # global t start
                hi = (n + 1) * 512
                w = hi - lo
                ps = pmm.tile([P, 512], f32, tag="mm", name="psmm")
                k = 0
                for zt, y in ((zre_t, yre), (zim_t, yim)):
                    for c in range(CT):
                        _mm(nc, ps[:, :w], y[c][j // 4][:, jsl],
                            zt[c][:, lo:hi],
                            start=(k == 0), stop=(k == 2 * CT - 1))
                        k += 1
                dst = pt[(j, n)]
                off = lo - n * 512
                if lo == j * P:
                    # diagonal block: mask t<u, then exp
                    fr = sp.tile([P, P], f32, tag="fr", name="fr")
                    nc.vector.tensor_add(out=fr, in0=ps[:, :P], in1=tril)
                    nc.scalar.activation(
                        out=dst[:, off:off + P], in_=fr,
                        func=mybir.ActivationFunctionType.Exp)
                    if w > P:
                        nc.scalar.activation(
                            out=dst[:, off + P:off + w], in_=ps[:, P:w],
                            func=mybir.ActivationFunctionType.Exp)
                else:
                    nc.scalar.activation(
                        out=dst[:, off:off + w], in_=ps[:, :w],
                        func=mybir.ActivationFunctionType.Exp)

            def emit_l_mm(lps, j, n, njs):
                """one colsum accumulation step for Pt[(j,n)]"""
                lo = max(0, j * P - n * 512)
                _mm(nc, lps[:, lo:512], onesc, pt[(j, n)][:, lo:512],
                    start=(j == 0), stop=(j == njs - 1))

            def emit_rlb(lps):
                """R = ones x (1/l): [P,512], staged to SBUF."""
                rl = sp.tile([1, 512], f32r, tag="rl", name="rl")
                with nc.allow_low_precision(reason="f32r is bit-identical f32"):
                    nc.vector.reciprocal(out=rl, in_=lps)
                rlb = pr.tile([P, 512], f32, tag="rlb", name="rlb")
                _mm(nc, rlb, onesr, rl, start=True, stop=True)
                # tensor_mul can read only one PSUM operand -> stage to SBUF
                rlb_sb = sp.tile([P, 512], f32, tag="rlbsb", name="rlb_sb",
                                 bufs=2)
                nc.vector.tensor_copy(out=rlb_sb, in_=rlb)
                return rlb_sb

            def emit_out_chunk(n, half, rlb=None, rlb_provider=None,
                               fine_dma=False):
                """out[:, n*512:(n+1)*512] for re (half 0) / im (half 1).

                rlb_provider: emitted after the first m-group's matmuls so
                the l/reciprocal chain hides under them.
                fine_dma: one DMA per m-tile (shorter tail exposure).
                """
                u, dram = ((ure, outre_d), (uim, outim_d))[half]
                tsl = slice(n * 512, (n + 1) * 512)
                js = list(range(min(4 * n + 4, 8) if n else 4))
                dview = dram.rearrange("(m p) t -> p m t", p=P)
                for mh in range(2):
                    gw = 1 if fine_dma else 2
                    for mi in range(2):
                        m = 2 * mh + mi
                        if mi % gw == 0:
                            o = wp.tile([P, gw, 512], bf16,
                                        tag=f"osb{gw}", name="osb")
                        msl = slice(m * P, (m + 1) * P)
                        ps = pmm.tile([P, 512], f32, tag="mm", name="psmm")
                        for j in js:
                            lo = max(0, j * P - n * 512)
                            _mm(nc, ps[:, lo:512],
                                u[j][:, msl], pt[(j, n)][:, lo:512],
                                start=(j == js[0]), stop=(j == js[-1]))
                        if rlb_provider is not None:
                            rlb = rlb_provider()
                            rlb_provider = None
                        # fused 1/l column scale during psum->sbuf
                        nc.vector.tensor_mul(out=o[:, mi % gw, :], in0=ps,
                                             in1=rlb)
                        if mi % gw == gw - 1:
                            mlo = m + 1 - gw
                            nc.sync.dma_start(
                                out=dview[:, mlo:m + 1, tsl], in_=o)

            # l-matmul for column j is emitted after scores-col j+1 so its
            # wait on exp(j) is always covered by independent matmuls
            lps0 = pl.tile([1, 512], f32, tag="l", name="lps")
            emit_scores_col(0, 0)
            emit_scores_col(1, 0)
            emit_l_mm(lps0, 0, 0, 4)
            emit_scores_col(2, 0)
            emit_l_mm(lps0, 1, 0, 4)
            emit_scores_col(3, 0)
            emit_l_mm(lps0, 2, 0, 4)
            emit_scores_col(0, 1)
            emit_l_mm(lps0, 3, 0, 4)
            emit_scores_col(1, 1)
            rlb0 = emit_rlb(lps0)
            emit_out_chunk(0, half=0, rlb=rlb0)
            emit_out_chunk(0, half=1, rlb=rlb0)
            lps1 = pl.tile([1, 512], f32, tag="l", name="lps")
            emit_scores_col(2, 1)
            emit_l_mm(lps1, 0, 1, 8)
            emit_scores_col(3, 1)
            emit_l_mm(lps1, 1, 1, 8)
            emit_scores_col(4, 1)
            emit_l_mm(lps1, 2, 1, 8)
            emit_scores_col(5, 1)
            emit_l_mm(lps1, 3, 1, 8)
            emit_scores_col(6, 1)
            emit_l_mm(lps1, 4, 1, 8)
            emit_scores_col(7, 1)
            emit_l_mm(lps1, 5, 1, 8)
            emit_l_mm(lps1, 6, 1, 8)

            rlb1_box = []

            def rlb1_provider():
                # emitted after out(1,h0)'s first m-group: the exp(7,1)
                # wait and the reciprocal hide under those matmuls
                emit_l_mm(lps1, 7, 1, 8)
                rlb1_box.append(emit_rlb(lps1))
                return rlb1_box[0]

            emit_out_chunk(1, half=0, rlb_provider=rlb1_provider)
            emit_out_chunk(1, half=1, rlb=rlb1_box[0], fine_dma=True)

    nc.compile()
    return nc


def _prep_weights(Wq, phi_q, Wk, phi_k, Wv, phi_v, Wo, phi_o):
    Wq, Wk, Wv, Wo = (np.asarray(w, np.float64) for w in (Wq, Wk, Wv, Wo))
    pq, pk, pv, po = (np.asarray(p, np.float64)
                      for p in (phi_q, phi_k, phi_v, phi_o))
    M = (Wq.T @ (np.exp(1j * (pk - pq))[:, None] * Wk)) / math.sqrt(DH)
    N = (np.exp(1j * po)[:, None] * Wo) @ (np.exp(1j * pv)[:, None] * Wv)
    has_imag = not (np.allclose(M.imag, 0.0) and np.allclose(N.imag, 0.0))
    return M, N, has_imag


def _consts(has_imag, M, N):
    import ml_dtypes
    snp = ml_dtypes.bfloat16
    consts = {
        "mtre": np.ascontiguousarray(M.real.T.astype(snp)),
        "ntre": np.ascontiguousarray(N.real.T.astype(snp)),
        "tril": np.tril(np.full((P, P), NEG, np.float32), -1),
        "onesc": np.ones((P, 1), snp),
        "onesr": np.ones((1, P), np.float32),
    }
    if has_imag:
        mtim = np.ascontiguousarray(M.imag.T.astype(snp))
        ntim = np.ascontiguousarray(N.imag.T.astype(snp))
        consts.update(mtim=mtim, mtimn=-mtim, ntim=ntim, ntimn=-ntim)
    return consts


def kernel(z_re, z_im, Wq, phi_q, Wk, phi_k, Wv, phi_v, Wo, phi_o):
    import ml_dtypes
    snp = ml_dtypes.bfloat16
    z_re = np.ascontiguousarray(np.asarray(z_re, np.float32).astype(snp))
    z_im = np.ascontiguousarray(np.asarray(z_im, np.float32).astype(snp))
    M, N, has_imag = _prep_weights(Wq, phi_q, Wk, phi_k, Wv, phi_v, Wo, phi_o)
    consts = _consts(has_imag, M, N)

    nc = _get_program(has_imag)
    in_maps = [
        dict(consts, zre=z_re[b].reshape(C, T), zim=z_im[b].reshape(C, T))
        for b in range(B)
    ]
    res = run_bass_kernel_spmd(nc, in_maps, list(range(B)))
    out_re = np.stack([np.asarray(res.results[b]["outre"], np.float32)
                       .reshape(C, HH, WW) for b in range(B)])
    out_im = np.stack([np.asarray(res.results[b]["outim"], np.float32)
                       .reshape(C, HH, WW) for b in range(B)])
    return out_re, out_im


# revision 30
# speedup vs baseline: 1.1225x; 1.0412x over previous
"""Trainium2 Bass kernel for nn_ComplexAttention (B=8, C=512, H=W=32, HEADS=8).

Strategy
--------
Data-parallel over batch: one batch element per NeuronCore (8 cores), no
collectives.  Host-side algebraic fusion shrinks the per-core work:

  reference:  Q = R_q Wq Z,  K = R_k Wk Z,  V = R_v Wv Z   (complex, [C,T])
              S = Re(Q^H K)/sqrt(dh),  causal softmax -> A
              out = R_o Wo (V A^T)

  fused:      M = Wq^T diag(e^{i(phi_k-phi_q)}) Wk / sqrt(dh)   (host, f64)
              N = diag(e^{i phi_o}) Wo diag(e^{i phi_v}) Wv     (host, f64)
              Y = M Z            (channel-major [C,T])
              S = Re(Z^H Y)      = Zre^T Yre + Zim^T Yim
              A = softmax(causal(S))        (no max-subtraction: |S| < ~30)
              U = N Z            (token-major [T,C])
              out = U^T A^T      (channel-major [C,T], = re/im pair)

Per-core tensor-engine work is ~320 [128x128x512] matmuls + 36 transposes,
all bf16 (1 cyc/row on the PE at any N, LDWEIGHTS at half the f32 cost,
half the DMA bytes); PSUM accumulates fp32.  End-to-end rel err ~7.8e-3
against the f64 oracle (budget 2e-2).

Schedule notes (from HW traces):
 - input DMA is BW-bound, so loads are interleaved with the first matmul
   phases (mtre+zre -> Y_re, ntre -> U_re, zim -> rest) on ONE sync
   queue (a second parallel queue steals HBM bandwidth from the critical
   first loads - measured).
 - softmax exp reads scores straight out of PSUM (no copy), per-chunk
   partial row-sums are added on DVE afterwards.
 - t-tiles 4..7 are processed first so the final out chunk (t 512..1023)
   overlaps the scores/softmax of t-tiles 0..3.
"""

import math

import numpy as np

import concourse.mybir as mybir
import concourse.tile as tile
from concourse import bacc
from concourse.bass_utils import run_bass_kernel_spmd

B, C, HH, WW = 8, 512, 32, 32
T = HH * WW          # 1024 tokens
DH = C // 8          # head dim (scale only)
P = 128
CT = C // P          # 4 channel tiles
TT = T // P          # 8 token tiles
NEG = -1.0e30
DIAG_SCALE = False   # PE transpose mode requires a permutation matrix

f32 = mybir.dt.float32
f32r = mybir.dt.float32r
bf16 = mybir.dt.bfloat16
VALUE_BF16 = True    # U / P / P^T path in bf16
FULL_BF16 = True     # scores path (Z, M, N, Y) in bf16 too (7.8e-3)


def _mm(nc, out, lhsT, rhs, start, stop):
    nc.tensor.matmul(out, lhsT, rhs, start=start, stop=stop)


_CACHE: dict = {}


def _get_program(has_imag: bool):
    key = has_imag
    if key not in _CACHE:
        _CACHE[key] = _build_program(has_imag)
    return _CACHE[key]


def _build_program(has_imag: bool):
    nc = bacc.Bacc("TRN2", target_bir_lowering=False, debug=False)

    sdt = bf16 if FULL_BF16 else f32r
    zre_d = nc.dram_tensor("zre", [C, T], sdt, kind="ExternalInput").ap()
    zim_d = nc.dram_tensor("zim", [C, T], sdt, kind="ExternalInput").ap()
    mtre_d = nc.dram_tensor("mtre", [C, C], sdt, kind="ExternalInput").ap()
    ntre_d = nc.dram_tensor("ntre", [C, C], sdt, kind="ExternalInput").ap()
    if has_imag:
        mtim_d = nc.dram_tensor("mtim", [C, C], sdt, kind="ExternalInput").ap()
        mtimn_d = nc.dram_tensor("mtimn", [C, C], sdt, kind="ExternalInput").ap()
        ntim_d = nc.dram_tensor("ntim", [C, C], sdt, kind="ExternalInput").ap()
        ntimn_d = nc.dram_tensor("ntimn", [C, C], sdt, kind="ExternalInput").ap()
    vdt = bf16 if VALUE_BF16 else f32r
    ident_d = nc.dram_tensor("ident", [P, P], vdt, kind="ExternalInput").ap()
    tri_d = nc.dram_tensor("tri", [P, P], f32, kind="ExternalInput").ap()
    trif_d = nc.dram_tensor("trif", [P, 256], f32, kind="ExternalInput").ap()
    zpad_d = nc.dram_tensor("zpad", [P, 384], vdt, kind="ExternalInput").ap()
    outre_d = nc.dram_tensor("outre", [C, T], f32, kind="ExternalOutput").ap()
    outim_d = nc.dram_tensor("outim", [C, T], f32, kind="ExternalOutput").ap()

    with tile.TileContext(nc) as tc:
        with (
            tc.tile_pool(name="const", bufs=1) as cp,
            tc.tile_pool(name="work", bufs=4) as wp,
            tc.tile_pool(name="small", bufs=12) as sp,
            tc.tile_pool(name="psmm", bufs=6, space="PSUM") as pmm,
            tc.tile_pool(name="pstr", bufs=2, space="PSUM") as ptr,
        ):
            def load_rows_on(dram, tag, eng):
                tiles = []
                for c in range(CT):
                    t = cp.tile([P, C], sdt, tag=f"{tag}{c}",
                                name=f"{tag}{c}")
                    eng.dma_start(out=t, in_=dram[c * P:(c + 1) * P, :])
                    tiles.append(t)
                return tiles

            def load_half(dram, tag, half, eng):
                tiles = []
                for c in range(CT):
                    t = cp.tile([P, 512], sdt, tag=f"{tag}{c}_{half}",
                                name=f"{tag}{c}_{half}")
                    eng.dma_start(
                        out=t,
                        in_=dram[c * P:(c + 1) * P,
                                 half * 512:(half + 1) * 512])
                    tiles.append(t)
                return tiles

            # -- small constants + first compute inputs ---------------------
            # single sync queue (HBM BW is shared; parallel queues starve
            # the critical first loads), ordered by first use, with mtre/zre
            # interleaved per c-tile so accumulation starts after ~0.5MB.
            ident = cp.tile([P, P], vdt, tag="ident", name="ident")
            nc.gpsimd.dma_start(out=ident, in_=ident_d)
            tri = cp.tile([P, P], f32, tag="tri", name="tri")
            nc.gpsimd.dma_start(out=tri, in_=tri_d)
            trif = cp.tile([P, 256], f32, tag="trif", name="trif")
            nc.gpsimd.dma_start(out=trif, in_=trif_d)
            mtre = [cp.tile([P, C], sdt, tag=f"mtre{c}", name=f"mtre{c}")
                    for c in range(CT)]
            zre_h = [[cp.tile([P, 512], sdt, tag=f"zre{c}_{h}",
                              name=f"zre{c}_{h}") for c in range(CT)]
                     for h in range(2)]

            def load_mtre_zre(c, h):
                if h == 0:
                    nc.sync.dma_start(out=mtre[c],
                                      in_=mtre_d[c * P:(c + 1) * P, :])
                nc.sync.dma_start(out=zre_h[h][c],
                                  in_=zre_d[c * P:(c + 1) * P,
                                            h * 512:(h + 1) * 512])

            # persistent result tiles (split by column half: precise deps)
            yre = [[cp.tile([P, 512], sdt, tag=f"yre{c}_{n}",
                            name=f"yre{c}_{n}") for n in range(2)]
                   for c in range(CT)]
            yim = [[cp.tile([P, 512], sdt, tag=f"yim{c}_{n}",
                            name=f"yim{c}_{n}") for n in range(2)]
                   for c in range(CT)]
            ure = [cp.tile([P, C], vdt, tag=f"ure{j}", name=f"ure{j}")
                   for j in range(TT)]
            uim = [cp.tile([P, C], vdt, tag=f"uim{j}", name=f"uim{j}")
                   for j in range(TT)]

            def psum_to_sbuf(dst_ap, src_ap):
                nc.vector.tensor_copy(out=dst_ap, in_=src_ap)

            def emit_y(dst, terms, load_hook=None):
                nterm = len(terms)
                for n in range(2):
                    pss = [pmm.tile([P, 512], f32, tag="mm", name="psmm")
                           for _ in range(CT)]
                    for t_i, (w, zh) in enumerate(terms):
                        for c in range(CT):
                            if load_hook is not None:
                                load_hook(c, n)
                            for m in range(CT):
                                _mm(nc, pss[m], w[c][:, m * P:(m + 1) * P],
                                    zh[n][c],
                                    start=(t_i == 0 and c == 0),
                                    stop=(t_i == nterm - 1 and c == CT - 1))
                    for m in range(CT):
                        psum_to_sbuf(dst[m][n], pss[m])

            def emit_u(dst, terms):
                for j in range(TT):
                    usl = slice((j % 4) * P, (j % 4 + 1) * P)
                    ps = pmm.tile([P, 512], f32, tag="mm", name="psmm")
                    nacc = len(terms) * CT
                    k = 0
                    for zh, w in terms:
                        for c in range(CT):
                            _mm(nc, ps, zh[j // 4][c][:, usl], w[c][:, :],
                                start=(k == 0), stop=(k == nacc - 1))
                            k += 1
                    psum_to_sbuf(dst[j], ps)

            # -- Y_re (needs mtre+zre only), then stream in the rest.
            # Later loads are EMITTED after emit_y so the watermark-style
            # sem waits on the first matmuls don't cover them; the DMA
            # engines still run their own streams immediately.
            if not has_imag:
                emit_y(yre, [(mtre, zre_h)], load_hook=load_mtre_zre)
                ntre = load_rows_on(ntre_d, "ntre", nc.sync)
                zim_h = [load_half(zim_d, "zim", 0, nc.sync),
                         load_half(zim_d, "zim", 1, nc.sync)]
                emit_u(ure, [(zre_h, ntre)])
                emit_y(yim, [(mtre, zim_h)])
                emit_u(uim, [(zim_h, ntre)])
            else:
                for c in range(CT):
                    load_mtre_zre(c, 0)
                for c in range(CT):
                    load_mtre_zre(c, 1)
                zim_h = [load_half(zim_d, "zim", 0, nc.sync),
                         load_half(zim_d, "zim", 1, nc.sync)]
                mtim = load_rows_on(mtim_d, "mtim", nc.sync)
                mtimn = load_rows_on(mtimn_d, "mtimn", nc.sync)
                ntre = load_rows_on(ntre_d, "ntre", nc.sync)
                ntim = load_rows_on(ntim_d, "ntim", nc.sync)
                ntimn = load_rows_on(ntimn_d, "ntimn", nc.sync)
                emit_y(yre, [(mtre, zre_h), (mtimn, zim_h)])
                emit_y(yim, [(mtre, zim_h), (mtim, zre_h)])
                emit_u(ure, [(zre_h, ntre), (zim_h, ntimn)])
                emit_u(uim, [(zim_h, ntre), (zre_h, ntim)])

            # -- P^T blocks (u-tile j, t-chunk n); zero upper regions -------
            pt = {}
            for j in range(TT):
                for n in range(2):
                    if n == 0 and j >= 4:
                        continue
                    ptile = cp.tile([P, 512], vdt, tag=f"pt{j}_{n}",
                                    name=f"pt{j}_{n}")
                    pt[(j, n)] = ptile
                    # the OUT clamp reads from col 256 even when the first
                    # transposed block starts later -> zero-fill the gap
                    lo = j * P - n * 512
                    if lo > 256:
                        nc.sync.dma_start(out=ptile[:, 256:lo],
                                          in_=zpad_d[:, 0:lo - 256])

            def emit_out_chunk(n, half=None, cols=(0, 512)):
                """out[:, n*512+cols] = U^T @ P^T for re and/or im."""
                c0, c1 = cols
                width = c1 - c0
                jmax = 4 * n + 3
                tsl = slice(n * 512 + c0, n * 512 + c1)
                pairs = ((ure, outre_d, nc.sync), (uim, outim_d, nc.sync))
                if half is not None:
                    pairs = (pairs[half],)
                js = [j for j in range(jmax + 1)
                      if max(c0, j * P - n * 512) < c1]
                for u, dram, oeng in pairs:
                    # two combined SBUF tiles -> two 512KB DMAs per half,
                    # so the transfer starts after 2 copies and the final
                    # post-compute drain is halved
                    dview = dram.rearrange("(m p) t -> p m t", p=P)
                    for mh in range(2):
                        o = wp.tile([P, 2, 512], f32, tag="osb", name="osb")
                        for mi in range(2):
                            m = 2 * mh + mi
                            msl = slice(m * P, (m + 1) * P)
                            ps = pmm.tile([P, 512], f32, tag="mm",
                                          name="psmm")
                            for j in js:
                                # pt[(j, n)] is all-zero left of column lo;
                                # clamp: N<256 f32r runs at 4 cyc/row
                                if FULL_BF16:
                                    lo = max(c0, j * P - n * 512)
                                else:
                                    lo = min(max(c0, j * P - n * 512),
                                             c1 - 256)
                                    lo = max(lo, c0)
                                _mm(nc, ps[:, lo - c0: width],
                                    u[j][:, msl], pt[(j, n)][:, lo:c1],
                                    start=(j == js[0]), stop=(j == js[-1]))
                            psum_to_sbuf(o[:, mi, :width], ps[:, :width])
                        oeng.dma_start(
                            out=dview[:, 2 * mh:2 * mh + 2, tsl],
                            in_=o[:, :, :width])

            # -- scores / softmax / transposes per t-tile -------------------
            def emit_scores_tile(i):
                ui = (i + 1) * P
                isl = slice((i % 4) * P, (i % 4 + 1) * P)
                s_sb = wp.tile([P, T], vdt, tag="s", name="s_sb")
                nchunks = (ui + 511) // 512
                lparts = []
                for q in range(nchunks):
                    w = min(512, ui - q * 512)
                    # widen 128-col chunks to 256: N<256 f32r matmuls run
                    # at 4 cyc/row, so the padded 256-col matmul is cheaper.
                    # Padded cols are masked to -inf -> exp 0.
                    wpad = w if FULL_BF16 else (
                        max(w, 256) if q == nchunks - 1 else w)
                    ps = pmm.tile([P, 512], f32, tag="mm", name="psmm")
                    k = 0
                    for zh, y in ((zre_h, yre), (zim_h, yim)):
                        for c in range(CT):
                            _mm(nc, ps[:, :wpad], zh[i // 4][c][:, isl],
                                y[c][q][:, :wpad],
                                start=(k == 0), stop=(k == 2 * CT - 1))
                            k += 1
                    last = q == nchunks - 1
                    if last:
                        fw = wpad - w + P   # frontier+pad width (128 or 256)
                        mask = tri if fw == P else trif
                        if wpad > fw:
                            # non-frontier part: exp straight from PSUM
                            lp = sp.tile([P, 1], f32, tag="lp", name="lp")
                            nc.scalar.activation(
                                out=s_sb[:, q * 512: q * 512 + wpad - fw],
                                in_=ps[:, : wpad - fw],
                                func=mybir.ActivationFunctionType.Exp,
                                accum_out=lp,
                            )
                            lparts.append(lp)
                        # frontier (+pad) cols: +mask (DVE), then exp
                        fr = sp.tile([P, 256], f32, tag="fr", name="fr")
                        nc.vector.tensor_add(out=fr[:, :fw],
                                             in0=ps[:, wpad - fw: wpad],
                                             in1=mask)
                        lp = sp.tile([P, 1], f32, tag="lp", name="lp")
                        nc.scalar.activation(
                            out=s_sb[:, ui - P: ui - P + fw],
                            in_=fr[:, :fw],
                            func=mybir.ActivationFunctionType.Exp,
                            accum_out=lp,
                        )
                        lparts.append(lp)
                    else:
                        lp = sp.tile([P, 1], f32, tag="lp", name="lp")
                        nc.scalar.activation(
                            out=s_sb[:, q * 512: q * 512 + w],
                            in_=ps[:, :w],
                            func=mybir.ActivationFunctionType.Exp,
                            accum_out=lp,
                        )
                        lparts.append(lp)

                lsum = lparts[0]
                for extra in lparts[1:]:
                    acc = sp.tile([P, 1], f32, tag="lacc", name="lacc")
                    nc.vector.tensor_add(out=acc, in0=lsum, in1=extra)
                    lsum = acc
                rl = sp.tile([P, 1], f32, tag="rl", name="rl")
                nc.vector.reciprocal(out=rl, in_=lsum)

                if DIAG_SCALE:
                    dg = sp.tile([P, P], f32r, tag="dg", name="dg")
                    nc.vector.tensor_scalar_mul(dg, ident, rl)
                    rhs = dg
                else:
                    nc.vector.tensor_scalar_mul(s_sb[:, :ui], s_sb[:, :ui],
                                                rl)
                    rhs = ident

                n = i // 4
                for j in range(i + 1):
                    pstile = ptr.tile([P, P], vdt, tag="tr", name="pstile")
                    nc.tensor.transpose(pstile, s_sb[:, j * P:(j + 1) * P],
                                        rhs)
                    nc.vector.tensor_copy(
                        out=pt[(j, n)][:, i * P - n * 512:
                                       (i + 1) * P - n * 512],
                        in_=pstile,
                    )

            for i in (4, 5, 6, 3):
                emit_scores_tile(i)
            emit_scores_tile(7)
            emit_scores_tile(2)
            emit_scores_tile(1)
            emit_out_chunk(1, half=0)
            emit_scores_tile(0)
            emit_out_chunk(1, half=1)
            emit_out_chunk(0, half=0)
            emit_out_chunk(0, half=1)

    nc.compile()
    return nc


def _prep_weights(Wq, phi_q, Wk, phi_k, Wv, phi_v, Wo, phi_o):
    Wq, Wk, Wv, Wo = (np.asarray(w, np.float64) for w in (Wq, Wk, Wv, Wo))
    pq, pk, pv, po = (np.asarray(p, np.float64)
                      for p in (phi_q, phi_k, phi_v, phi_o))
    M = (Wq.T @ (np.exp(1j * (pk - pq))[:, None] * Wk)) / math.sqrt(DH)
    N = (np.exp(1j * po)[:, None] * Wo) @ (np.exp(1j * pv)[:, None] * Wv)
    has_imag = not (np.allclose(M.imag, 0.0) and np.allclose(N.imag, 0.0))
    return M, N, has_imag


def _consts(has_imag, M, N):
    import ml_dtypes
    snp = ml_dtypes.bfloat16 if FULL_BF16 else np.float32
    vnp = ml_dtypes.bfloat16 if VALUE_BF16 else np.float32
    consts = {
        "mtre": np.ascontiguousarray(M.real.T.astype(snp)),
        "ntre": np.ascontiguousarray(N.real.T.astype(snp)),
        "ident": np.eye(P, dtype=vnp),
        "tri": np.triu(np.full((P, P), NEG, np.float32), 1),
        "trif": np.concatenate(
            [np.triu(np.full((P, P), NEG, np.float32), 1),
             np.full((P, P), NEG, np.float32)], axis=1),
        "zpad": np.zeros((P, 384), vnp),
    }
    if has_imag:
        mtim = np.ascontiguousarray(M.imag.T.astype(snp))
        ntim = np.ascontiguousarray(N.imag.T.astype(snp))
        consts.update(mtim=mtim, mtimn=-mtim, ntim=ntim, ntimn=-ntim)
    return consts


def kernel(z_re, z_im, Wq, phi_q, Wk, phi_k, Wv, phi_v, Wo, phi_o):
    import ml_dtypes
    snp = ml_dtypes.bfloat16 if FULL_BF16 else np.float32
    z_re = np.ascontiguousarray(np.asarray(z_re, np.float32).astype(snp))
    z_im = np.ascontiguousarray(np.asarray(z_im, np.float32).astype(snp))
    M, N, has_imag = _prep_weights(Wq, phi_q, Wk, phi_k, Wv, phi_v, Wo, phi_o)
    consts = _consts(has_imag, M, N)

    nc = _get_program(has_imag)
    in_maps = [
        dict(consts, zre=z_re[b].reshape(C, T), zim=z_im[b].reshape(C, T))
        for b in range(B)
    ]
    res = run_bass_kernel_spmd(nc, in_maps, list(range(B)))
    out_re = np.stack([np.asarray(res.results[b]["outre"], np.float32)
                       .reshape(C, HH, WW) for b in range(B)])
    out_im = np.stack([np.asarray(res.results[b]["outim"], np.float32)
                       .reshape(C, HH, WW) for b in range(B)])
    return out_re, out_im
